# revision 20
# baseline (speedup 1.0000x reference)
"""CTC batch loss kernel for Trainium2 (8 NeuronCores, batch-parallel).

Math: reference computes logp = log_softmax(log(y+eps)) = log(y+eps) - log(rowsum).
We run the DP in probability space with periodic renormalization, split into a
FORWARD chain (alpha, t=1..TSTAR) and a BACKWARD chain (beta, t=255..TSTAR+1)
that meet at TSTAR. Emissions are pre-divided BY THE HOST by u_blank(t) (the
blank emission), which turns the blank-state updates into pure adds; the
division cancels in the final log-correction:
  loss[b] = sum_t log rs'(t) - sum_r log m_r - log(sum alpha~*beta~)
where rs'(t) = rowsum(y/ub)(t) = rs(t)/ub(t).

Per-core layout (32 samples/core):
  - y_pred divided by ub, cast bf16, transposed on host to
    [group(4 samples), q, c(part 128), j(4), cchunk(8), t(64)]; ybf DMAs are
    batched 4 samples at a time on the gpsimd queue, a full quarter
    prefetched ahead.
  - One-hot matrix O_b [1024, 130] per sample (host, fp8e4 - exact for 0/1):
    lanes [su(64) | ul(64) | ones | pad]; su = skip-masked ul; ones lane =
    rowsum. All 8 group DMAs issued up-front on the scalar queue.
  - PE accumulates over 8 c-chunks per (sample, quarter); ACT copies
    PSUM->SBUF bf16; SBUF->SBUF DMA repacks [64t,130] into
    em_q[32b, 64t, 130], alternating between the sync and gpsimd queues.
  - Produce order q0, q3, q1, q2 matches DP consumption so the DP starts as
    soon as em0 lands and never long-stalls on a quarter.
  - DP on the Vector engine: solo fwd steps t=1..63 (em0 only), then fwd/bwd
    pairs interleaved [f1,b1,f2,b2,f3,b3] so each dependent pair is >= 2
    slots apart and DVE latency is hidden.
      fwd: E(65)=alpha_even, MW=[0|o(64)|..|o2(64)|..|E0|E1|q]:
        f1: [E_new|q] = E_old(x2) + [(0,o)|(o,0)];
        f3: XX = [(0,o)*su' | q*ul'];  f2: o = XX[0:64]+XX[64:128]
      bwd: BE(65), BO(64), G=[h(64)|0|g_o(64)|0]:
        b1: G = [BO*su' | BO*ul'];  b2: T2 = BE[1:65]+G[1:65];
        b3: [BE_new|BO] = [BE_cur|T2]+[g_o,0](x2)
  - Renorm by max every 16 steps per chain (+1 late fwd renorm pre-merge).
  - sum_t log rs' accumulated on the Scalar engine (Ln with accum_out)
    straight from the em tiles' ones lane; no Vector-side prep at all.
"""

import math
import os
import sys
from contextlib import ExitStack

import numpy as np

sys.path.insert(0, "/opt/trn_rl_repo")
sys.path.insert(0, "/root/.axon_site/_ro/trn_rl_repo")

import ml_dtypes  # noqa: E402

B, T, C, L = 256, 256, 1024, 64
NCORES = 8
BS = B // NCORES  # 32 samples per core
NG = BS // 4  # 8 groups of 4 samples
NLANE = 130  # 64 su | 64 ul | ones | pad
ONES_LANE = 2 * L
KCH = C // 128  # 8 contraction chunks
NQ = 4
TQW = T // NQ  # 64
NORM_EVERY = 16
SOLO_F = 63  # solo fwd steps t=1..63 (quarter 0 only)
TSTAR = 159  # fwd computes alpha(1..TSTAR); bwd beta via t=255..TSTAR+1
NPAIRS = 255 - TSTAR  # 96 interleaved fwd/bwd pairs
FWD_RENORMS = list(range(NORM_EVERY - 1, TSTAR - 1, NORM_EVERY)) + [TSTAR - 1]
NNF = len(FWD_RENORMS)  # 6: t = 31,63,95,127,143?? recomputed below
NNB = (255 - (TSTAR + 1)) // NORM_EVERY + 1  # bwd renorms at bi=31,63,95 -> 3
BLANK = C - 1
EPS = 1e-7


# ---------------------------------------------------------------- host prep

def host_prep_y(y_pred: np.ndarray) -> np.ndarray:
    """[B, T, C] f32 -> [B/4, NQ, 128(c part), 4(j), KCH, TQW(t)] bf16.

    Divides by the blank emission (y[..,BLANK]+eps) so blank lanes become 1
    on-chip; the division cancels in the final log-correction via rs'.
    """
    y = y_pred + np.float32(EPS)
    y /= y[:, :, BLANK:BLANK + 1]
    # b = g*4 + j ; t = q*64 + tt ; c = k*128 + cc
    yt = y.reshape(B // 4, 4, NQ, TQW, KCH, 128).transpose(0, 2, 5, 1, 4, 3)
    return np.ascontiguousarray(yt).astype(ml_dtypes.bfloat16)


def host_prep_oh(y_true: np.ndarray) -> np.ndarray:
    """[B, L] int -> one-hot+aux matrix [B/4 groups, 128(c part), 4, KCH, NLANE]."""
    lab = y_true.astype(np.int64)
    oh = np.zeros((B, C, NLANE), dtype=np.float32)
    bidx = np.arange(B)[:, None]
    jidx = np.arange(L)[None, :]
    skip = np.zeros((B, L), dtype=np.float32)
    skip[:, 1:] = (lab[:, 1:] != lab[:, :-1]).astype(np.float32)
    oh[bidx, lab, jidx] = skip  # su lanes (first!)
    oh[bidx, lab, jidx + L] = 1.0  # ul lanes
    oh[:, :, ONES_LANE] = 1.0  # ones lane (rowsum)
    oh = oh.reshape(B // 4, 4, KCH, 128, NLANE).transpose(0, 3, 1, 2, 4)
    return np.ascontiguousarray(oh).astype(ml_dtypes.float8_e4m3)


# ---------------------------------------------------------------- bass build

def build_nc():
    import concourse.bass as bass
    import concourse.tile as tile
    from concourse import bacc, mybir

    f32 = mybir.dt.float32
    bf16 = mybir.dt.bfloat16
    f8 = mybir.dt.float8e4

    nc = bacc.Bacc(None, target_bir_lowering=False)

    yt_d = nc.declare_dram_parameter(
        "yt", [NG, NQ, 128, 4, KCH, TQW], bf16, isOutput=False
    )
    oh_d = nc.declare_dram_parameter(
        "oh", [NG, 128, 4, KCH, NLANE], f8, isOutput=False
    )
    out_d = nc.declare_dram_parameter("out", [BS, 1], f32, isOutput=True)

    with tile.TileContext(nc) as tc:
        with ExitStack() as ctx:
            ohp = ctx.enter_context(tc.tile_pool(name="ohp", bufs=1))
            yp = ctx.enter_context(tc.tile_pool(name="yp", bufs=8))
            psp = ctx.enter_context(
                tc.tile_pool(name="psp", bufs=8, space=bass.MemorySpace.PSUM)
            )
            stp = ctx.enter_context(tc.tile_pool(name="stp", bufs=16))
            emp = ctx.enter_context(tc.tile_pool(name="emp", bufs=1))
            alp = ctx.enter_context(tc.tile_pool(name="alp", bufs=1))
            fin = ctx.enter_context(tc.tile_pool(name="fin", bufs=1))

            # persistent DP state in mega-tiles addressed by 2-segment APs.
            # MW (fwd): 0 pad | o_c1@1(64) | pads | o_c2@67(64) | pad131 |
            #           E0@134(65) | E1@200(65) | q@266(64, col330 scratch)
            # BW (bwd): BE0@0(65) | BE1@66(65) | T2@132(64+scratch) | BO@198(64)
            # G  (bwd): h@0(64) | pads | g_o@66(64) | pads (132 wide)
            OC1, OC2, E0, E1, Q = 1, 67, 134, 200, 266
            BE0, BE1, T2O, BOO = 0, 66, 132, 198
            MW = alp.tile([BS, 532], bf16, name="mw")
            BW = alp.tile([BS, 396], bf16, name="bw")
            G = alp.tile([BS, 132], bf16, name="g")
            XX = alp.tile([BS, 2 * L], bf16, name="xx")
            NRM = fin.tile([BS, NNF + NNB], f32)
            TMPM = alp.tile([BS, 1], f32, name="tmpm")
            TMPR = alp.tile([BS, 1], f32, name="tmpr")

            def seg2(tile_, off1, off2, width):
                d = off2 - off1
                return tile_[:, off1 : off1 + 2 * d].rearrange(
                    "p (a b) -> p a b", a=2, b=d
                )[:, :, 0:width]

            for t_ in (MW, BW, G, XX):
                nc.vector.memset(t_[:], 0.0)
            nc.vector.memset(MW[:, E0 : E0 + 1], 1.0)  # e~(0) = [1,0..]
            nc.vector.memset(BW[:, BE0 + L : BE0 + L + 1], 1.0)  # be[64]=1
            nc.vector.memset(BW[:, BOO + L - 1 : BOO + L], 1.0)  # bo[63]=1

            em_sb = {}
            oh_sb = {}
            acc_rq = {}

            def load_oh(g):
                ohg = ohp.tile([128, 4, KCH, NLANE], f8, tag=f"oh{g}", name=f"oh{g}")
                oh_sb[g] = ohg
                nc.scalar.dma_start(ohg[:], oh_d[g])

            def produce(q, first=False):
                em = emp.tile([BS, TQW, NLANE], bf16, tag=f"em{q}", name=f"em{q}")
                em_sb[q] = em
                if first:
                    for g in range(NG):
                        load_oh(g)
                # prefetch the whole quarter's y up front (8 group DMAs)
                ybfs = []
                for g in range(NG):
                    ybf = yp.tile([128, 4, KCH, TQW], bf16, tag="ybf", name="ybf")
                    nc.gpsimd.dma_start(ybf[:], yt_d[g, q])
                    ybfs.append(ybf)
                for g in range(NG):
                    for j in range(4):
                        b = g * 4 + j
                        ps = psp.tile([TQW, NLANE], f32, tag="ps", name="ps")
                        for k in range(KCH):
                            nc.tensor.matmul(
                                ps[:], ybfs[g][:, j, k, :], oh_sb[g][:, j, k, :],
                                start=(k == 0), stop=(k == KCH - 1),
                            )
                        st = stp.tile([TQW, NLANE], bf16, tag="st", name="st")
                        nc.scalar.copy(st[:], ps[:])
                        # quarter 0 entirely on the gpsimd queue (the sync
                        # queue trails it by ~10us and em0 gates the DP start)
                        if q == 0 or b % 2 == 1:
                            nc.gpsimd.dma_start(em[b : b + 1], st[:])
                        else:
                            nc.sync.dma_start(em[b : b + 1], st[:])
                # sum_t log rs' for this quarter, on the Scalar engine
                scr = fin.tile([BS, TQW], f32, tag=f"scr{q}", name=f"scr{q}")
                acc = fin.tile([BS, 1], f32, tag=f"acc{q}", name=f"acc{q}")
                acc_rq[q] = acc
                nc.scalar.activation(
                    scr[:], em[:, :, ONES_LANE], mybir.ActivationFunctionType.Ln,
                    accum_out=acc[:],
                )

            def renorm(wide, a65, bscale, r):
                nc.vector.tensor_reduce(
                    NRM[:, r : r + 1], wide, mybir.AxisListType.X,
                    mybir.AluOpType.max,
                )
                nc.vector.reciprocal(TMPR[:], NRM[:, r : r + 1])
                nc.vector.tensor_scalar_mul(a65, a65, TMPR[:])
                nc.vector.tensor_scalar_mul(bscale, bscale, TMPR[:])

            def femit(t, phase):
                """Forward step t, op index phase (0..2)."""
                em = em_sb[t // TQW]
                tt = t % TQW
                ecur = E0 if (t - 1) % 2 == 0 else E1
                enew = E1 if ecur == E0 else E0
                if phase == 0:
                    # F12: [E_new | q] = E_old(x2) + [(0,o) | (o,0)]
                    out = seg2(MW, enew, Q, L + 1)
                    in0 = MW[:, ecur : ecur + L + 1][:, None, :].broadcast_to(
                        [BS, 2, L + 1]
                    )
                    in1 = seg2(MW, 0, OC2, L + 1)
                    nc.vector.tensor_add(out, in0, in1)
                elif phase == 1:
                    # XX = [(0,o)*su' | q*ul']
                    in0 = seg2(MW, 0, Q, L)
                    em2 = em[:, tt, 0 : 2 * L].rearrange(
                        "p (a b) -> p a b", a=2, b=L
                    )
                    nc.vector.tensor_mul(
                        XX[:, 0 : 2 * L].rearrange("p (a b) -> p a b", a=2, b=L),
                        in0, em2,
                    )
                else:
                    # o (both copies) = x1 + x2
                    out = seg2(MW, OC1, OC2, L)
                    x1 = XX[:, L : 2 * L][:, None, :].broadcast_to([BS, 2, L])
                    x2 = XX[:, 0:L][:, None, :].broadcast_to([BS, 2, L])
                    nc.vector.tensor_add(out, x1, x2)
                    if t in FWD_RENORMS:
                        renorm(
                            MW[:, 0:331],
                            MW[:, enew : enew + L + 1],
                            seg2(MW, OC1, OC2, L),
                            FWD_RENORMS.index(t),
                        )

            def bemit(t, phase):
                """Backward step consuming emissions at t, op index phase (0..2)."""
                em = em_sb[t // TQW]
                tt = t % TQW
                bi = 255 - t
                becur = BE0 if bi % 2 == 0 else BE1
                benew = BE1 if becur == BE0 else BE0
                if phase == 0:
                    # G = [BO*su' | BO*ul'] -> [h | g_o]
                    g2 = seg2(G, 0, 66, L)
                    bo2 = BW[:, BOO : BOO + L][:, None, :].broadcast_to([BS, 2, L])
                    em2 = em[:, tt, 0 : 2 * L].rearrange(
                        "p (a b) -> p a b", a=2, b=L
                    )
                    nc.vector.tensor_mul(g2, bo2, em2)
                elif phase == 1:
                    # T2 = BE[1:65] + h[j+1]   (G[1:65] = [h1..h63, 0])
                    nc.vector.tensor_add(
                        BW[:, T2O : T2O + L], BW[:, becur + 1 : becur + L + 1],
                        G[:, 1 : L + 1],
                    )
                else:
                    # [BE_new | BO] = [BE_cur | T2] + [g_o,0](x2)
                    out = seg2(BW, benew, BOO, L + 1)
                    in0 = seg2(BW, becur, T2O, L + 1)
                    in1 = G[:, 66 : 66 + L + 1][:, None, :].broadcast_to(
                        [BS, 2, L + 1]
                    )
                    nc.vector.tensor_add(out, in0, in1)
                    if bi % NORM_EVERY == NORM_EVERY - 1:
                        renorm(
                            BW[:, 0:262],
                            BW[:, benew : benew + L + 1],
                            BW[:, BOO : BOO + L],
                            NNF + bi // NORM_EVERY,
                        )

            # ---- emission + DP schedule ----
            produce(0, first=True)
            # init: o~(0)[0] = ul'(0)[0] (ul lanes start at L)
            nc.vector.tensor_copy(MW[:, OC1 : OC1 + 1], em_sb[0][:, 0, L : L + 1])
            nc.vector.tensor_copy(MW[:, OC2 : OC2 + 1], em_sb[0][:, 0, L : L + 1])
            for t in range(1, SOLO_F + 1):
                for ph in range(3):
                    femit(t, ph)
            produce(3)
            produce(1)
            produce(2)
            for i in range(NPAIRS):
                ft = SOLO_F + 1 + i
                bt = 255 - i
                for ph in range(3):
                    femit(ft, ph)
                    bemit(bt, ph)

            # ---- merge at TSTAR: L~ = sum(E*BE) + sum(o*BO)
            M1 = fin.tile([BS, L + 1], f32)
            M2 = fin.tile([BS, L], f32)
            R1 = fin.tile([BS, 1], f32)
            LS = fin.tile([BS, 1], f32)
            efin = E0 if TSTAR % 2 == 0 else E1
            befin = BE0 if (255 - TSTAR) % 2 == 0 else BE1
            nc.vector.tensor_mul(
                M1[:], MW[:, efin : efin + L + 1], BW[:, befin : befin + L + 1]
            )
            nc.vector.tensor_mul(M2[:], MW[:, OC1 : OC1 + L], BW[:, BOO : BOO + L])
            nc.vector.tensor_reduce(
                R1[:], M1[:], mybir.AxisListType.X, mybir.AluOpType.add
            )
            nc.vector.tensor_reduce(
                LS[:], M2[:], mybir.AxisListType.X, mybir.AluOpType.add
            )
            nc.vector.tensor_add(LS[:], LS[:], R1[:])
            ln_ls = fin.tile([BS, 1], f32)
            nc.scalar.activation(ln_ls[:], LS[:], mybir.ActivationFunctionType.Ln)
            scr_n = fin.tile([BS, NNF + NNB], f32)
            acc_n = fin.tile([BS, 1], f32)
            nc.scalar.activation(
                scr_n[:], NRM[:], mybir.ActivationFunctionType.Ln,
                scale=float(2.0 ** -16), accum_out=acc_n[:]
            )
            # loss = acc_r - acc_n - ln_ls
            loss = fin.tile([BS, 1], f32)
            nc.vector.tensor_add(loss[:], acc_rq[0][:], acc_rq[1][:])
            nc.vector.tensor_add(loss[:], loss[:], acc_rq[2][:])
            nc.vector.tensor_add(loss[:], loss[:], acc_rq[3][:])
            nc.vector.tensor_sub(loss[:], loss[:], acc_n[:])
            nc.vector.tensor_sub(loss[:], loss[:], ln_ls[:])
            # acc_n used Ln(m * 2^-16); add back (NNF+NNB)*16*ln2
            nc.vector.tensor_single_scalar(
                loss[:], loss[:], float((NNF + NNB) * 16.0 * math.log(2.0)),
                mybir.AluOpType.subtract,
            )
            nc.sync.dma_start(out_d[:], loss[:])

    nc.compile()
    return nc


_NC_CACHE = {}


def _get_nc():
    if "nc" not in _NC_CACHE:
        _NC_CACHE["nc"] = build_nc()
    return _NC_CACHE["nc"]


# ---------------------------------------------------------------- entrypoint

def kernel(y_true: np.ndarray, y_pred: np.ndarray, _trace: bool = False):
    from concourse.bass_utils import run_bass_kernel_spmd

    yt = host_prep_y(np.asarray(y_pred, dtype=np.float32))
    oh = host_prep_oh(np.asarray(y_true))

    in_maps = []
    for i in range(NCORES):
        sl = slice(i * NG, (i + 1) * NG)
        in_maps.append({"yt": yt[sl], "oh": oh[sl]})

    nc = _get_nc()
    res = run_bass_kernel_spmd(nc, in_maps, list(range(NCORES)), trace=_trace)
    out = np.concatenate([res.results[i]["out"] for i in range(NCORES)], axis=0)
    if _trace:
        return out.astype(np.float32), res
    return out.astype(np.float32)


# revision 21
# speedup vs baseline: 1.0212x; 1.0212x over previous
"""CTC batch loss kernel for Trainium2 (8 NeuronCores, batch-parallel).

Math: reference computes logp = log_softmax(log(y+eps)) = log(y+eps) - log(rowsum).
We run the DP in probability space with periodic renormalization, split into a
FORWARD chain (alpha, t=1..TSTAR) and a BACKWARD chain (beta, t=255..TSTAR+1)
that meet at TSTAR. Emissions are pre-divided BY THE HOST by u_blank(t) (the
blank emission), which turns the blank-state updates into pure adds; the
division cancels in the final log-correction:
  loss[b] = sum_t log rs'(t) - sum_r log m_r - log(sum alpha~*beta~)
where rs'(t) = rowsum(y/ub)(t) = rs(t)/ub(t).

Per-core layout (32 samples/core):
  - y_pred divided by ub, cast bf16, transposed on host to
    [group(4 samples), q, c(part 128), j(4), cchunk(8), t(64)]; ybf DMAs are
    batched 4 samples at a time on the gpsimd queue, a full quarter
    prefetched ahead.
  - One-hot matrix O_b [1024, 130] per sample (host, fp8e4 - exact for 0/1):
    lanes [su(64) | ul(64) | ones | pad]; su = skip-masked ul; ones lane =
    rowsum. All 8 group DMAs issued up-front on the scalar queue.
  - PE accumulates over 8 c-chunks per (sample, quarter); ACT copies
    PSUM->SBUF bf16; SBUF->SBUF DMA repacks [64t,130] into
    em_q[32b, 64t, 130], alternating between the sync and gpsimd queues.
  - Produce order q0, q3, q1, q2 matches DP consumption so the DP starts as
    soon as em0 lands and never long-stalls on a quarter.
  - DP on the Vector engine: solo fwd steps t=1..63 (em0 only), then fwd/bwd
    pairs interleaved [f1,b1,f2,b2,f3,b3] so each dependent pair is >= 2
    slots apart and DVE latency is hidden.
      fwd: E(65)=alpha_even, MW=[0|o(64)|..|o2(64)|..|E0|E1|q]:
        f1: [E_new|q] = E_old(x2) + [(0,o)|(o,0)];
        f3: XX = [(0,o)*su' | q*ul'];  f2: o = XX[0:64]+XX[64:128]
      bwd: BE(65), BO(64), G=[h(64)|0|g_o(64)|0]:
        b1: G = [BO*su' | BO*ul'];  b2: T2 = BE[1:65]+G[1:65];
        b3: [BE_new|BO] = [BE_cur|T2]+[g_o,0](x2)
  - Renorm by max every 16 steps per chain (+1 late fwd renorm pre-merge).
  - sum_t log rs' accumulated on the Scalar engine (Ln with accum_out)
    straight from the em tiles' ones lane; no Vector-side prep at all.
"""

import math
import os
import sys
from contextlib import ExitStack

import numpy as np

sys.path.insert(0, "/opt/trn_rl_repo")
sys.path.insert(0, "/root/.axon_site/_ro/trn_rl_repo")

import ml_dtypes  # noqa: E402

B, T, C, L = 256, 256, 1024, 64
NCORES = 8
BS = B // NCORES  # 32 samples per core
NG = BS // 4  # 8 groups of 4 samples
NLANE = 130  # 64 su | 64 ul | ones | pad
ONES_LANE = 2 * L
KCH = C // 128  # 8 contraction chunks
NQ = 4
TQW = T // NQ  # 64
NORM_EVERY = 16
SOLO_F = 63  # solo fwd steps t=1..63 (quarter 0 only)
TSTAR = 159  # fwd computes alpha(1..TSTAR); bwd beta via t=255..TSTAR+1
NPAIRS = 255 - TSTAR  # 96 interleaved fwd/bwd pairs
FWD_RENORMS = list(range(NORM_EVERY - 1, TSTAR - 1, NORM_EVERY)) + [TSTAR - 1]
NNF = len(FWD_RENORMS)  # 6: t = 31,63,95,127,143?? recomputed below
NNB = (255 - (TSTAR + 1)) // NORM_EVERY + 1  # bwd renorms at bi=31,63,95 -> 3
BLANK = C - 1
EPS = 1e-7


# ---------------------------------------------------------------- host prep

def host_prep_y(y_pred: np.ndarray) -> np.ndarray:
    """[B, T, C] f32 -> [B/4, NQ, 128(c part), 4(j), KCH, TQW(t)] bf16.

    Divides by the blank emission (y[..,BLANK]+eps) so blank lanes become 1
    on-chip; the division cancels in the final log-correction via rs'.
    """
    y = y_pred + np.float32(EPS)
    y /= y[:, :, BLANK:BLANK + 1]
    # b = g*4 + j ; t = q*64 + tt ; c = k*128 + cc
    yt = y.reshape(B // 4, 4, NQ, TQW, KCH, 128).transpose(0, 2, 5, 1, 4, 3)
    return np.ascontiguousarray(yt).astype(ml_dtypes.bfloat16)


def host_prep_oh(y_true: np.ndarray) -> np.ndarray:
    """[B, L] int -> one-hot+aux matrix [B/4 groups, 128(c part), 4, KCH, NLANE]."""
    lab = y_true.astype(np.int64)
    oh = np.zeros((B, C, NLANE), dtype=np.float32)
    bidx = np.arange(B)[:, None]
    jidx = np.arange(L)[None, :]
    skip = np.zeros((B, L), dtype=np.float32)
    skip[:, 1:] = (lab[:, 1:] != lab[:, :-1]).astype(np.float32)
    oh[bidx, lab, jidx] = skip  # su lanes (first!)
    oh[bidx, lab, jidx + L] = 1.0  # ul lanes
    oh[:, :, ONES_LANE] = 1.0  # ones lane (rowsum)
    oh = oh.reshape(B // 4, 4, KCH, 128, NLANE).transpose(0, 3, 1, 2, 4)
    return np.ascontiguousarray(oh).astype(ml_dtypes.float8_e4m3)


# ---------------------------------------------------------------- bass build

def build_nc():
    import concourse.bass as bass
    import concourse.tile as tile
    from concourse import bacc, mybir

    f32 = mybir.dt.float32
    bf16 = mybir.dt.bfloat16
    f8 = mybir.dt.float8e4

    nc = bacc.Bacc(None, target_bir_lowering=False)

    yt_d = nc.declare_dram_parameter(
        "yt", [NG, NQ, 128, 4, KCH, TQW], bf16, isOutput=False
    )
    oh_d = nc.declare_dram_parameter(
        "oh", [NG, 128, 4, KCH, NLANE], f8, isOutput=False
    )
    out_d = nc.declare_dram_parameter("out", [BS, 1], f32, isOutput=True)

    with tile.TileContext(nc) as tc:
        with ExitStack() as ctx:
            ohp = ctx.enter_context(tc.tile_pool(name="ohp", bufs=1))
            yp = ctx.enter_context(tc.tile_pool(name="yp", bufs=8))
            psp = ctx.enter_context(
                tc.tile_pool(name="psp", bufs=8, space=bass.MemorySpace.PSUM)
            )
            stp = ctx.enter_context(tc.tile_pool(name="stp", bufs=16))
            emp = ctx.enter_context(tc.tile_pool(name="emp", bufs=1))
            alp = ctx.enter_context(tc.tile_pool(name="alp", bufs=1))
            fin = ctx.enter_context(tc.tile_pool(name="fin", bufs=1))

            # persistent DP state in mega-tiles addressed by 2-segment APs.
            # MW (fwd): 0 pad | o_c1@1(64) | pads | o_c2@67(64) | pad131 |
            #           E0@134(65) | E1@200(65) | q@266(64, col330 scratch)
            # BW (bwd): BE0@0(65) | BE1@66(65) | T2@132(64+scratch) | BO@198(64)
            # G  (bwd): h@0(64) | pads | g_o@66(64) | pads (132 wide)
            OC1, OC2, E0, E1, Q = 1, 67, 134, 200, 266
            BE0, BE1, T2O, BOO = 0, 66, 132, 198
            MW = alp.tile([BS, 532], bf16, name="mw")
            BW = alp.tile([BS, 396], bf16, name="bw")
            G = alp.tile([BS, 132], bf16, name="g")
            XX = alp.tile([BS, 2 * L], bf16, name="xx")
            NRM = fin.tile([BS, NNF + NNB], f32)
            TMPM = alp.tile([BS, 1], f32, name="tmpm")
            TMPR = alp.tile([BS, 1], f32, name="tmpr")

            def seg2(tile_, off1, off2, width):
                d = off2 - off1
                return tile_[:, off1 : off1 + 2 * d].rearrange(
                    "p (a b) -> p a b", a=2, b=d
                )[:, :, 0:width]

            for t_ in (MW, BW, G, XX):
                nc.vector.memset(t_[:], 0.0)
            nc.vector.memset(MW[:, E0 : E0 + 1], 1.0)  # e~(0) = [1,0..]
            nc.vector.memset(BW[:, BE0 + L : BE0 + L + 1], 1.0)  # be[64]=1
            nc.vector.memset(BW[:, BOO + L - 1 : BOO + L], 1.0)  # bo[63]=1

            em_sb = {}
            oh_sb = {}
            acc_rq = {}

            def load_oh(g):
                ohg = ohp.tile([128, 4, KCH, NLANE], f8, tag=f"oh{g}", name=f"oh{g}")
                oh_sb[g] = ohg
                nc.scalar.dma_start(ohg[:], oh_d[g])

            def produce(q, first=False):
                em = emp.tile([BS, TQW, NLANE], bf16, tag=f"em{q}", name=f"em{q}")
                em_sb[q] = em
                if first:
                    for g in range(NG):
                        load_oh(g)
                # prefetch the whole quarter's y up front (8 group DMAs)
                ybfs = []
                for g in range(NG):
                    ybf = yp.tile([128, 4, KCH, TQW], bf16, tag="ybf", name="ybf")
                    nc.gpsimd.dma_start(ybf[:], yt_d[g, q])
                    ybfs.append(ybf)
                for g in range(NG):
                    for j in range(4):
                        b = g * 4 + j
                        ps = psp.tile([TQW, NLANE], f32, tag="ps", name="ps")
                        for k in range(KCH):
                            nc.tensor.matmul(
                                ps[:], ybfs[g][:, j, k, :], oh_sb[g][:, j, k, :],
                                start=(k == 0), stop=(k == KCH - 1),
                            )
                        st = stp.tile([TQW, NLANE], bf16, tag="st", name="st")
                        nc.scalar.copy(st[:], ps[:])
                        eng = nc.sync if b % 2 == 0 else nc.gpsimd
                        eng.dma_start(em[b : b + 1], st[:])
                # sum_t log rs' for this quarter, on the Scalar engine
                scr = fin.tile([BS, TQW], f32, tag=f"scr{q}", name=f"scr{q}")
                acc = fin.tile([BS, 1], f32, tag=f"acc{q}", name=f"acc{q}")
                acc_rq[q] = acc
                nc.scalar.activation(
                    scr[:], em[:, :, ONES_LANE], mybir.ActivationFunctionType.Ln,
                    accum_out=acc[:],
                )

            def renorm(wide, a65, bscale, r):
                nc.vector.tensor_reduce(
                    NRM[:, r : r + 1], wide, mybir.AxisListType.X,
                    mybir.AluOpType.max,
                )
                nc.vector.reciprocal(TMPR[:], NRM[:, r : r + 1])
                nc.vector.tensor_scalar_mul(a65, a65, TMPR[:])
                nc.vector.tensor_scalar_mul(bscale, bscale, TMPR[:])

            def femit(t, phase):
                """Forward step t, op index phase (0..2)."""
                em = em_sb[t // TQW]
                tt = t % TQW
                ecur = E0 if (t - 1) % 2 == 0 else E1
                enew = E1 if ecur == E0 else E0
                if phase == 0:
                    # F12: [E_new | q] = E_old(x2) + [(0,o) | (o,0)]
                    out = seg2(MW, enew, Q, L + 1)
                    in0 = MW[:, ecur : ecur + L + 1][:, None, :].broadcast_to(
                        [BS, 2, L + 1]
                    )
                    in1 = seg2(MW, 0, OC2, L + 1)
                    nc.vector.tensor_add(out, in0, in1)
                elif phase == 1:
                    # XX = [(0,o)*su' | q*ul']
                    in0 = seg2(MW, 0, Q, L)
                    em2 = em[:, tt, 0 : 2 * L].rearrange(
                        "p (a b) -> p a b", a=2, b=L
                    )
                    nc.vector.tensor_mul(
                        XX[:, 0 : 2 * L].rearrange("p (a b) -> p a b", a=2, b=L),
                        in0, em2,
                    )
                else:
                    # o (both copies) = x1 + x2
                    out = seg2(MW, OC1, OC2, L)
                    x1 = XX[:, L : 2 * L][:, None, :].broadcast_to([BS, 2, L])
                    x2 = XX[:, 0:L][:, None, :].broadcast_to([BS, 2, L])
                    nc.vector.tensor_add(out, x1, x2)
                    if t in FWD_RENORMS:
                        renorm(
                            MW[:, 0:331],
                            MW[:, enew : enew + L + 1],
                            seg2(MW, OC1, OC2, L),
                            FWD_RENORMS.index(t),
                        )

            def bemit(t, phase):
                """Backward step consuming emissions at t, op index phase (0..2)."""
                em = em_sb[t // TQW]
                tt = t % TQW
                bi = 255 - t
                becur = BE0 if bi % 2 == 0 else BE1
                benew = BE1 if becur == BE0 else BE0
                if phase == 0:
                    # G = [BO*su' | BO*ul'] -> [h | g_o]
                    g2 = seg2(G, 0, 66, L)
                    bo2 = BW[:, BOO : BOO + L][:, None, :].broadcast_to([BS, 2, L])
                    em2 = em[:, tt, 0 : 2 * L].rearrange(
                        "p (a b) -> p a b", a=2, b=L
                    )
                    nc.vector.tensor_mul(g2, bo2, em2)
                elif phase == 1:
                    # T2 = BE[1:65] + h[j+1]   (G[1:65] = [h1..h63, 0])
                    nc.vector.tensor_add(
                        BW[:, T2O : T2O + L], BW[:, becur + 1 : becur + L + 1],
                        G[:, 1 : L + 1],
                    )
                else:
                    # [BE_new | BO] = [BE_cur | T2] + [g_o,0](x2)
                    out = seg2(BW, benew, BOO, L + 1)
                    in0 = seg2(BW, becur, T2O, L + 1)
                    in1 = G[:, 66 : 66 + L + 1][:, None, :].broadcast_to(
                        [BS, 2, L + 1]
                    )
                    nc.vector.tensor_add(out, in0, in1)
                    if bi % NORM_EVERY == NORM_EVERY - 1:
                        renorm(
                            BW[:, 0:262],
                            BW[:, benew : benew + L + 1],
                            BW[:, BOO : BOO + L],
                            NNF + bi // NORM_EVERY,
                        )

            # ---- emission + DP schedule ----
            produce(0, first=True)
            # init: o~(0)[0] = ul'(0)[0] (ul lanes start at L)
            nc.vector.tensor_copy(MW[:, OC1 : OC1 + 1], em_sb[0][:, 0, L : L + 1])
            nc.vector.tensor_copy(MW[:, OC2 : OC2 + 1], em_sb[0][:, 0, L : L + 1])
            for t in range(1, SOLO_F + 1):
                for ph in range(3):
                    femit(t, ph)
            produce(3)
            produce(1)
            produce(2)
            for i in range(NPAIRS):
                ft = SOLO_F + 1 + i
                bt = 255 - i
                for ph in range(3):
                    femit(ft, ph)
                    bemit(bt, ph)

            # ---- merge at TSTAR: L~ = sum(E*BE) + sum(o*BO)
            M1 = fin.tile([BS, L + 1], f32)
            M2 = fin.tile([BS, L], f32)
            R1 = fin.tile([BS, 1], f32)
            LS = fin.tile([BS, 1], f32)
            efin = E0 if TSTAR % 2 == 0 else E1
            befin = BE0 if (255 - TSTAR) % 2 == 0 else BE1
            nc.vector.tensor_mul(
                M1[:], MW[:, efin : efin + L + 1], BW[:, befin : befin + L + 1]
            )
            nc.vector.tensor_mul(M2[:], MW[:, OC1 : OC1 + L], BW[:, BOO : BOO + L])
            nc.vector.tensor_reduce(
                R1[:], M1[:], mybir.AxisListType.X, mybir.AluOpType.add
            )
            nc.vector.tensor_reduce(
                LS[:], M2[:], mybir.AxisListType.X, mybir.AluOpType.add
            )
            nc.vector.tensor_add(LS[:], LS[:], R1[:])
            ln_ls = fin.tile([BS, 1], f32)
            nc.scalar.activation(ln_ls[:], LS[:], mybir.ActivationFunctionType.Ln)
            scr_n = fin.tile([BS, NNF + NNB], f32)
            acc_n = fin.tile([BS, 1], f32)
            nc.scalar.activation(
                scr_n[:], NRM[:], mybir.ActivationFunctionType.Ln,
                scale=float(2.0 ** -16), accum_out=acc_n[:]
            )
            # loss = acc_r - acc_n - ln_ls
            loss = fin.tile([BS, 1], f32)
            nc.vector.tensor_add(loss[:], acc_rq[0][:], acc_rq[1][:])
            nc.vector.tensor_add(loss[:], loss[:], acc_rq[2][:])
            nc.vector.tensor_add(loss[:], loss[:], acc_rq[3][:])
            nc.vector.tensor_sub(loss[:], loss[:], acc_n[:])
            nc.vector.tensor_sub(loss[:], loss[:], ln_ls[:])
            # acc_n used Ln(m * 2^-16); add back (NNF+NNB)*16*ln2
            nc.vector.tensor_single_scalar(
                loss[:], loss[:], float((NNF + NNB) * 16.0 * math.log(2.0)),
                mybir.AluOpType.subtract,
            )
            nc.sync.dma_start(out_d[:], loss[:])

    nc.compile()
    return nc


_NC_CACHE = {}


def _get_nc():
    if "nc" not in _NC_CACHE:
        _NC_CACHE["nc"] = build_nc()
    return _NC_CACHE["nc"]


# ---------------------------------------------------------------- entrypoint

def kernel(y_true: np.ndarray, y_pred: np.ndarray, _trace: bool = False):
    from concourse.bass_utils import run_bass_kernel_spmd

    yt = host_prep_y(np.asarray(y_pred, dtype=np.float32))
    oh = host_prep_oh(np.asarray(y_true))

    in_maps = []
    for i in range(NCORES):
        sl = slice(i * NG, (i + 1) * NG)
        in_maps.append({"yt": yt[sl], "oh": oh[sl]})

    nc = _get_nc()
    res = run_bass_kernel_spmd(nc, in_maps, list(range(NCORES)), trace=_trace)
    out = np.concatenate([res.results[i]["out"] for i in range(NCORES)], axis=0)
    if _trace:
        return out.astype(np.float32), res
    return out.astype(np.float32)
